# revision 23
# baseline (speedup 1.0000x reference)
"""Trainium2 Bass kernel for nn_AutoregressiveHDFormer.

Model: B=4, L=1024, D=DFF=512, VOCAB=32000, 2 enc + 2 dec layers,
sign-based attention (softmax over the query axis, masks applied after
softmax), final 512x32000 projection.

Sharding (8 cores, zero cross-core communication): core c computes the
full embed+encoder+decoder stack for batch b = c % 4 (cores c and c+4
duplicate stage 1), then writes vocab half h = c // 4 of the final
projection for that batch. The host only shards inputs and reassembles
the output.

On-chip layout: activations are feature-major float32r
(xT: [128 part, 4 feature k-tiles, 1024 tokens]); attention runs in
(key i, query j) space where the softmax over j is a free-axis
reduction; LN statistics over the feature axis use ones-matmul column
reductions broadcast across partitions. Sign tensors are exact bf16.

Softmax details vs the reference:
- logits = (sk.sq + |sq|^2_col)/1024; the |sq|^2 column term equals D
  exactly unless a projection output is exactly 0.0 (measure-zero), and
  softmax over j is invariant to j-constant shifts, so it is dropped.
- the 1/1024 scale folds into the Exp activation scale.
- the softmax 1/sum folds into the V tensor (per-key scaling commutes
  with the attn^T @ v contraction over keys).
"""
import numpy as np

import concourse.bacc as bacc
import concourse.bass as bass
import concourse.mybir as mybir
import concourse.tile as tile
from concourse.bass_utils import run_bass_kernel_spmd
from concourse.masks import make_identity

fp32 = mybir.dt.float32
f32r = mybir.dt.float32r
bf16 = mybir.dt.bfloat16
i32 = mybir.dt.int32
AF = mybir.ActivationFunctionType
ALU = mybir.AluOpType
AX = mybir.AxisListType
ts = bass.ts

B, L, D, DFF, VOCAB = 4, 1024, 512, 512, 32000
NL = 2
KT = D // 128            # 4 feature k-tiles
NT = L // 128            # 8 token tiles
EPS = 1e-6
LOGIT_DIV = 1024.0
SQRT_D = float(np.sqrt(np.float32(D)))
VH = VOCAB // 2          # vocab half per core
VCH = 500                # vocab chunk (<=512: one PSUM bank)
NVCH = VH // VCH

_cached = {}


# ---------------------------------------------------------------- host prep

def _np(x):
    return np.asarray(x, dtype=np.float32)


def _pos_encoding():
    pos = np.arange(L, dtype=np.float32)[:, None]
    i = np.arange(D, dtype=np.float32)[None, :]
    rates = (1.0 / np.power(np.float32(10000.0),
                            (2.0 * np.floor(i / 2.0) / np.float32(D))
                            .astype(np.float32))).astype(np.float32)
    ang = (pos * rates).astype(np.float32)
    pe = np.empty((L, D), np.float32)
    pe[:, 0::2] = np.sin(ang[:, 0::2])
    pe[:, 1::2] = np.cos(ang[:, 1::2])
    return pe


def _ktile(w):
    """(D_in, N) -> [KT, 128, N] contraction-tiled layout."""
    din, n = w.shape
    return np.ascontiguousarray(w.reshape(din // 128, 128, n)
                                .astype(np.float32))


class _H:
    """attribute bag"""


# ---------------------------------------------------------------- build

def build_bass():
    nc = bacc.Bacc()

    tok_enc = nc.dram_tensor("tok_enc", [1, L], i32, kind="ExternalInput")
    tok_dec = nc.dram_tensor("tok_dec", [1, L], i32, kind="ExternalInput")
    emb_enc = nc.dram_tensor("emb_enc", [VOCAB, D], fp32, kind="ExternalInput")
    emb_dec = nc.dram_tensor("emb_dec", [VOCAB, D], fp32, kind="ExternalInput")
    pe_d = nc.dram_tensor("pe", [L, D], fp32, kind="ExternalInput")
    tri_mask = nc.dram_tensor("tri_mask", [L, L], fp32, kind="ExternalInput")
    wts = {}
    for side in ("e", "d"):
        nmha = 1 if side == "e" else 2
        for li in range(NL):
            for mi in range(nmha):
                pre = f"{side}{li}m{mi}"
                for w in ("wq", "wk", "wv", "wo"):
                    wts[pre + w] = nc.dram_tensor(pre + w, [KT, 128, D], f32r,
                                                  kind="ExternalInput")
                wts[pre + "anc"] = nc.dram_tensor(pre + "anc", [KT, 128, L],
                                                  fp32, kind="ExternalInput")
            wts[f"{side}{li}w1"] = nc.dram_tensor(f"{side}{li}w1",
                                                  [KT, 128, DFF], f32r,
                                                  kind="ExternalInput")
            wts[f"{side}{li}w2"] = nc.dram_tensor(f"{side}{li}w2",
                                                  [KT, 128, DFF], f32r,
                                                  kind="ExternalInput")
    fw_d = nc.dram_tensor("fw", [KT, 128, VH], f32r, kind="ExternalInput")
    out_d = nc.dram_tensor("out", [L, VH], fp32, kind="ExternalOutput")

    with tile.TileContext(nc) as tc:
        with tc.tile_pool(name="sb", bufs=1) as sb, \
             tc.tile_pool(name="psum", bufs=1, space="PSUM") as psp:
            _body(nc, sb, psp, tok_enc, tok_dec, emb_enc, emb_dec, pe_d,
                  tri_mask, wts, fw_d, out_d)
    nc.finalize()
    return nc


def _body(nc, sb, psp, tok_enc, tok_dec, emb_enc, emb_dec, pe_d, tri_mask,
          wts, fw_d, out_d):
    C = _H()
    # ---------------- constants
    C.onesf = sb.tile([128, 128], fp32, tag="c_onesf")
    nc.vector.memset(C.onesf, 1.0)
    C.ones_r = sb.tile([128, 128], f32r, tag="c_onesr")
    nc.vector.tensor_copy(out=C.ones_r[:, :], in_=C.onesf[:, :])
    C.ones_b = sb.tile([128, 128], bf16, tag="c_onesb")
    nc.vector.memset(C.ones_b, 1.0)
    C.eps = sb.tile([128, 1], fp32, tag="c_eps")
    nc.vector.memset(C.eps, EPS)
    C.one = sb.tile([128, 1], fp32, tag="c_one")
    nc.vector.memset(C.one, 1.0)
    C.ident = sb.tile([128, 128], fp32, tag="c_ident")
    make_identity(nc, C.ident)

    def act_tile(name, tag="act", bufs=2):
        return sb.tile([128, KT, L], f32r, tag=tag, bufs=bufs, name=name)

    # ---------------- embedding: gather + scale + posenc -> feature-major
    def embed(tok_dram, table_dram, name):
        tokt = sb.tile([128, NT], i32, tag="tok", bufs=2)
        nc.sync.dma_start(out=tokt[:, :],
                          in_=tok_dram[0, :].rearrange("(n p) -> p n", p=128))
        xT = act_tile(name)
        for n in range(NT):
            g = sb.tile([128, D], fp32, tag="emb_g", bufs=1)
            nc.gpsimd.indirect_dma_start(
                out=g[:, :], out_offset=None, in_=table_dram[:, :],
                in_offset=bass.IndirectOffsetOnAxis(ap=tokt[:, n:n + 1],
                                                    axis=0))
            pe_t = sb.tile([128, D], fp32, tag="emb_pe", bufs=1)
            nc.sync.dma_start(out=pe_t[:, :], in_=pe_d[ts(n, 128), :])
            x0 = sb.tile([128, D], fp32, tag="emb_x0", bufs=1)
            nc.vector.scalar_tensor_tensor(out=x0[:, :], in0=g[:, :],
                                           scalar=SQRT_D, in1=pe_t[:, :],
                                           op0=ALU.mult, op1=ALU.add)
            for k in range(KT):
                tp = psp.tile([128, 512], fp32, tag="pA", bufs=4)
                nc.tensor.transpose(tp[:, :128], x0[:, ts(k, 128)],
                                    C.ident[:, :])
                nc.vector.tensor_copy(out=xT[:, k, ts(n, 128)],
                                      in_=tp[:, :128])
        return xT

    # ---------------- encoder query-axis mask, broadcast to 128 partitions
    def enc_mask_bcast(tok_dram):
        row_i = sb.tile([1, L], i32, tag="mrow_i")
        nc.sync.dma_start(out=row_i[:, :], in_=tok_dram[:, :])
        row_f = sb.tile([1, L], fp32, tag="mrow_f")
        nc.vector.tensor_copy(out=row_f[:, :], in_=row_i[:, :])
        row_m = sb.tile([1, L], f32r, tag="mrow_m")
        nc.scalar.activation(row_m[:, :], row_f[:, :], AF.Relu,
                             scale=-1.0, bias=C.one[:1, :])
        mb_ps = psp.tile([128, L], fp32, tag="pL", bufs=2)
        for jc in range(2):
            nc.tensor.matmul(mb_ps[:, ts(jc, 512)], C.ones_r[:1, :],
                             row_m[:, ts(jc, 512)], start=True, stop=True)
        mb = sb.tile([128, L], fp32, tag="c_mb")
        nc.vector.tensor_copy(out=mb[:, :], in_=mb_ps[:, :])
        return mb

    # ---------------- MHA (feature-major in/out; adds residual in place)
    def mha(xq, kv, pre, mask_dram, mask_bcast, resid):
        """xq/kv: [128,KT,L] f32r. Writes resid += attn_out; returns None.

        mask_dram: (L,L) fp32 key x query mask, or None.
        mask_bcast: (128,L) fp32 query mask bcast, or None.
        resid: tile updated in place: resid = resid + mha_out (f32r).
        """
        # q/k sign projections (feature-major)
        sqT = sb.tile([128, KT, L], bf16, tag="sq", name=pre + "sq")
        skT = sb.tile([128, KT, L], bf16, tag="sk", name=pre + "sk")
        for (wname, src, dst) in ((pre + "wq", xq, sqT), (pre + "wk", kv, skT)):
            w_t = sb.tile([128, KT, D], f32r, tag="w", bufs=2, name="w_t")
            nc.sync.dma_start(out=w_t[:, :, :],
                              in_=wts[wname][:, :, :].rearrange(
                                  "k p n -> p k n"))
            for dt_ in range(KT):
                for jc in range(2):
                    ps = psp.tile([128, 512], fp32, tag="pA", bufs=4,
                                  name="ps_qk")
                    for k in range(KT):
                        nc.tensor.matmul(ps[:, :], w_t[:, k, ts(dt_, 128)],
                                         src[:, k, ts(jc, 512)],
                                         start=(k == 0), stop=(k == KT - 1))
                    nc.scalar.activation(dst[:, dt_, ts(jc, 512)], ps[:, :],
                                         AF.Sign)
        # logits -> softmax stats -> masked exp (f32r attn), rec saved
        attn = sb.tile([128, NT, L], f32r, tag="attn", name=pre + "attn")
        rec_all = sb.tile([128, NT], fp32, tag="rec", bufs=2, name="rec")
        for it in range(NT):
            lg = psp.tile([128, L], fp32, tag="pL", bufs=2, name="lg")
            for jc in range(2):
                for k in range(KT):
                    nc.tensor.matmul(lg[:, ts(jc, 512)],
                                     skT[:, k, ts(it, 128)],
                                     sqT[:, k, ts(jc, 512)],
                                     start=(k == 0), stop=(k == KT - 1))
            mx = sb.tile([128, 1], fp32, tag="row", bufs=4, name="mx")
            nc.vector.tensor_reduce(out=mx[:, :], in_=lg[:, :], axis=AX.X,
                                    op=ALU.max, negate=True)
            mxs = sb.tile([128, 1], fp32, tag="row", bufs=4, name="mxs")
            nc.vector.tensor_scalar_mul(mxs[:, :], mx[:, :], 1.0 / LOGIT_DIV)
            # exp written straight into the attn tile; sum before masking
            nc.scalar.activation(attn[:, it, :], lg[:, :], AF.Exp,
                                 bias=mxs[:, :], scale=1.0 / LOGIT_DIV)
            sm = sb.tile([128, 1], fp32, tag="row", bufs=4, name="sm")
            nc.vector.tensor_reduce(out=sm[:, :],
                                    in_=attn[:, it, :].bitcast(fp32),
                                    axis=AX.X, op=ALU.add)
            nc.vector.reciprocal(out=rec_all[:, it:it + 1], in_=sm[:, :])
            if mask_dram is not None:
                mk = sb.tile([128, L], fp32, tag="msk", bufs=2, name="mk")
                nc.sync.dma_start(out=mk[:, :],
                                  in_=mask_dram[ts(it, 128), :])
                nc.vector.tensor_mul(attn[:, it, :],
                                     attn[:, it, :].bitcast(fp32), mk[:, :])
            else:
                nc.vector.tensor_mul(attn[:, it, :],
                                     attn[:, it, :].bitcast(fp32),
                                     mask_bcast[:, :])
        # v (token-major) with 1/sum folded in
        v_sb = sb.tile([128, NT, D], f32r, tag="v", name=pre + "v")
        w_t = sb.tile([128, KT, D], f32r, tag="w", bufs=2, name="w_t")
        nc.sync.dma_start(out=w_t[:, :, :],
                          in_=wts[pre + "wv"][:, :, :].rearrange(
                              "k p n -> p k n"))
        for it in range(NT):
            ps = psp.tile([128, 512], fp32, tag="pA", bufs=4, name="ps_v")
            for k in range(KT):
                nc.tensor.matmul(ps[:, :], kv[:, k, ts(it, 128)],
                                 w_t[:, k, :],
                                 start=(k == 0), stop=(k == KT - 1))
            nc.vector.tensor_scalar_mul(v_sb[:, it, :], ps[:, :],
                                        rec_all[:, it:it + 1])
        # attnV -> anchor*tanh -> u
        u = sb.tile([128, KT, L], f32r, tag="u", name=pre + "u")
        for dt_ in range(KT):
            for jc in range(2):
                ps = psp.tile([128, 512], fp32, tag="pA", bufs=4,
                              name="ps_av")
                for it in range(NT):
                    nc.tensor.matmul(ps[:, :], v_sb[:, it, ts(dt_, 128)],
                                     attn[:, it, ts(jc, 512)],
                                     start=(it == 0), stop=(it == NT - 1))
                an = sb.tile([128, 512], fp32, tag="anc", bufs=2, name="an")
                nc.sync.dma_start(out=an[:, :],
                                  in_=wts[pre + "anc"][dt_, :, ts(jc, 512)])
                tt = sb.tile([128, 512], fp32, tag="half", bufs=2, name="tt")
                nc.vector.tensor_mul(tt[:, :], ps[:, :], an[:, :])
                nc.scalar.activation(u[:, dt_, ts(jc, 512)], tt[:, :],
                                     AF.Tanh)
        # wo projection + residual (in place into resid)
        w_t = sb.tile([128, KT, D], f32r, tag="w", bufs=2, name="w_t")
        nc.sync.dma_start(out=w_t[:, :, :],
                          in_=wts[pre + "wo"][:, :, :].rearrange(
                              "k p n -> p k n"))
        for dt_ in range(KT):
            for jc in range(2):
                ps = psp.tile([128, 512], fp32, tag="pA", bufs=4,
                              name="ps_wo")
                for k in range(KT):
                    nc.tensor.matmul(ps[:, :], w_t[:, k, ts(dt_, 128)],
                                     u[:, k, ts(jc, 512)],
                                     start=(k == 0), stop=(k == KT - 1))
                nc.vector.tensor_add(resid[:, dt_, ts(jc, 512)],
                                     resid[:, dt_, ts(jc, 512)].bitcast(fp32),
                                     ps[:, :])

    # ---------------- LayerNorm over features (feature-major, f32r out)
    def ln_feat(src, name, dst_tag="act"):
        """src: [128,KT,L] f32r tile -> new act tile (normalized)."""
        s_ps = psp.tile([128, L], fp32, tag="pL", bufs=2, name="s_ps")
        q_ps = psp.tile([128, L], fp32, tag="pL", bufs=2, name="q_ps")
        for jc in range(2):
            sl = ts(jc, 512)
            for k in range(KT):
                nc.tensor.matmul(s_ps[:, sl], C.ones_r[:, :], src[:, k, sl],
                                 start=(k == 0), stop=(k == KT - 1))
            for k in range(KT):
                sq = sb.tile([128, 512], f32r, tag="half_r", bufs=2,
                             name="sqr")
                nc.scalar.activation(sq[:, :], src[:, k, sl].bitcast(fp32),
                                     AF.Square)
                nc.tensor.matmul(q_ps[:, sl], C.ones_r[:, :], sq[:, :],
                                 start=(k == 0), stop=(k == KT - 1))
        mean = sb.tile([128, L], fp32, tag="stat", bufs=2, name="mean")
        nc.vector.tensor_scalar_mul(mean[:, :], s_ps[:, :], 1.0 / D)
        var = sb.tile([128, L], fp32, tag="stat", bufs=2, name="var")
        # var = q/D - mean^2 : first -mean^2, then add q/D
        nc.vector.scalar_tensor_tensor(out=var[:, :], in0=mean[:, :],
                                       scalar=-1.0, in1=mean[:, :],
                                       op0=ALU.mult, op1=ALU.mult)
        nc.vector.scalar_tensor_tensor(out=var[:, :], in0=q_ps[:, :],
                                       scalar=1.0 / D, in1=var[:, :],
                                       op0=ALU.mult, op1=ALU.add)
        # in place: var -> sd -> rstd ; mean -> mean*rstd
        nc.scalar.activation(var[:, :], var[:, :], AF.Sqrt, bias=C.eps[:, :])
        nc.vector.reciprocal(out=var[:, :], in_=var[:, :])
        nc.vector.tensor_mul(mean[:, :], mean[:, :], var[:, :])
        y = act_tile(name, tag=dst_tag, bufs=1 if dst_tag != "act" else 2)
        for k in range(KT):
            nc.vector.tensor_mul(y[:, k, :], src[:, k, :].bitcast(fp32),
                                 var[:, :])
            nc.vector.tensor_sub(y[:, k, :], y[:, k, :].bitcast(fp32),
                                 mean[:, :])
        return y

    def ffn(y, pre):
        w_t = sb.tile([128, KT, DFF], f32r, tag="w", bufs=2, name="w_t")
        nc.sync.dma_start(out=w_t[:, :, :],
                          in_=wts[pre + "w1"][:, :, :].rearrange(
                              "k p n -> p k n"))
        h = sb.tile([128, KT, L], f32r, tag="u", name=pre + "h")
        for dt_ in range(KT):
            for jc in range(2):
                ps = psp.tile([128, 512], fp32, tag="pA", bufs=4,
                              name="ps_f1")
                for k in range(KT):
                    nc.tensor.matmul(ps[:, :], w_t[:, k, ts(dt_, 128)],
                                     y[:, k, ts(jc, 512)],
                                     start=(k == 0), stop=(k == KT - 1))
                nc.scalar.activation(h[:, dt_, ts(jc, 512)], ps[:, :],
                                     AF.Relu)
        w_t2 = sb.tile([128, KT, DFF], f32r, tag="w", bufs=2, name="w_t")
        nc.sync.dma_start(out=w_t2[:, :, :],
                          in_=wts[pre + "w2"][:, :, :].rearrange(
                              "k p n -> p k n"))
        f = act_tile(pre + "f")
        for dt_ in range(KT):
            for jc in range(2):
                ps = psp.tile([128, 512], fp32, tag="pA", bufs=4,
                              name="ps_f2")
                for k in range(KT):
                    nc.tensor.matmul(ps[:, :], w_t2[:, k, ts(dt_, 128)],
                                     h[:, k, ts(jc, 512)],
                                     start=(k == 0), stop=(k == KT - 1))
                nc.vector.tensor_copy(out=f[:, dt_, ts(jc, 512)],
                                      in_=ps[:, :])
        return f

    # ================================================= encoder
    m_enc = enc_mask_bcast(tok_enc)
    xT = embed(tok_enc, emb_enc, "xTe")
    for li in range(NL):
        mha(xT, xT, f"e{li}m0", None, m_enc, resid=xT)
        y = ln_feat(xT, f"e{li}y")
        f = ffn(y, f"e{li}")
        last = (li == NL - 1)
        xT = ln_feat(f, f"e{li}x",
                     dst_tag="enc_keep" if last else "act")
    encT = xT

    # ================================================= decoder
    yT = embed(tok_dec, emb_dec, "xTd")
    for li in range(NL):
        mha(yT, yT, f"d{li}m0", tri_mask, None, resid=yT)
        y1 = ln_feat(yT, f"d{li}y1")
        mha(y1, encT, f"d{li}m1", tri_mask, None, resid=y1)
        y2 = ln_feat(y1, f"d{li}y2")
        f = ffn(y2, f"d{li}")
        yT = ln_feat(f, f"d{li}x")

    # ================================================= final projection
    for ch in range(NVCH):
        fw_t = sb.tile([128, KT, VCH], f32r, tag="w", bufs=2, name="fw_t")
        nc.sync.dma_start(
            out=fw_t[:, :, :],
            in_=fw_d[:, :, :].rearrange("k p v -> p k v")[:, :,
                                                          ts(ch, VCH)])
        for tt in range(NT):
            ps = psp.tile([128, 512], fp32, tag="pA", bufs=4, name="ps_fw")
            for k in range(KT):
                nc.tensor.matmul(ps[:, :VCH], yT[:, k, ts(tt, 128)],
                                 fw_t[:, k, :],
                                 start=(k == 0), stop=(k == KT - 1))
            ob = sb.tile([128, VCH], fp32, tag="ob", bufs=3, name="ob")
            nc.vector.tensor_copy(out=ob[:, :], in_=ps[:, :VCH])
            nc.sync.dma_start(out=out_d[ts(tt, 128), ts(ch, VCH)],
                              in_=ob[:, :])


# ---------------------------------------------------------------- entry

def kernel(inp, tar, params):
    inp = np.asarray(inp).astype(np.int32)
    tar = np.asarray(tar).astype(np.int32)

    pe = _pos_encoding()
    tri = (1.0 - np.tril(np.ones((L, L), np.float32))).astype(np.float32)

    base = {
        "pe": pe, "tri_mask": tri,
        "emb_enc": _np(params["emb_enc"]),
        "emb_dec": _np(params["emb_dec"]),
    }
    for side, lays in (("e", params["enc_layers"]),
                       ("d", params["dec_layers"])):
        for li, p in enumerate(lays):
            mhas = [p["mha"]] if side == "e" else [p["mha1"], p["mha2"]]
            for mi, m in enumerate(mhas):
                pre = f"{side}{li}m{mi}"
                for w in ("wq", "wk", "wv", "wo"):
                    base[pre + w] = _ktile(_np(m[w]))
                base[pre + "anc"] = _ktile(_np(m["anchor"]).T)
                for bn in ("bq", "bk", "bv", "bo"):
                    assert not np.any(_np(m[bn])), f"nonzero {bn}"
            for gn in [k for k in p if k.endswith("_g")]:
                assert np.all(_np(p[gn]) == 1.0), f"non-unit {gn}"
            for bn in [k for k in p if k.endswith("_b")]:
                assert not np.any(_np(p[bn])), f"nonzero {bn}"
            assert not np.any(_np(p["b1"])) and not np.any(_np(p["b2"]))
            base[f"{side}{li}w1"] = _ktile(_np(p["w1"]))
            base[f"{side}{li}w2"] = _ktile(_np(p["w2"]))
    assert not np.any(_np(params["fb"])), "nonzero fb"
    fw = _np(params["fw"])

    if _cached.get("nc") is None:
        _cached["nc"] = build_bass()

    in_maps = []
    for c in range(8):
        b, h = c % 4, c // 4
        m = dict(base)
        m["tok_enc"] = inp[b:b + 1]
        m["tok_dec"] = tar[b:b + 1]
        m["fw"] = np.ascontiguousarray(
            fw[:, h * VH:(h + 1) * VH].reshape(KT, 128, VH))
        in_maps.append(m)

    _cached["in_maps"] = in_maps
    res = run_bass_kernel_spmd(_cached["nc"], in_maps,
                               core_ids=list(range(8)))
    _cached["last_res"] = res
    out = np.empty((B, L, VOCAB), np.float32)
    for c in range(8):
        b, h = c % 4, c // 4
        out[b, :, h * VH:(h + 1) * VH] = res.results[c]["out"]
    return out


def bench_exec_time(iters=5):
    """Median wall time of one sharded device execution (inputs pre-staged).

    Mimics bass2jax.run_bass_via_pjrt's execute step; includes per-call
    dispatch overhead but not input upload.
    """
    import time
    import jax
    from jax.sharding import Mesh, PartitionSpec, NamedSharding
    from jax.experimental.shard_map import shard_map
    from concourse import bass2jax, mybir as _mybir
    from concourse.bass2jax import _bass_exec_p, partition_id_tensor

    nc = _cached["nc"]
    in_maps = _cached["in_maps"]
    n_cores = 8
    partition_name = (nc.partition_id_tensor.name
                      if nc.partition_id_tensor else None)
    in_names, out_names, out_avals, zero_outs = [], [], [], []
    for alloc in nc.m.functions[0].allocations:
        if not isinstance(alloc, _mybir.MemoryLocationSet):
            continue
        name = alloc.memorylocations[0].name
        if alloc.kind == "ExternalInput":
            if name != partition_name:
                in_names.append(name)
        elif alloc.kind == "ExternalOutput":
            out_names.append(name)
            shape = tuple(alloc.tensor_shape)
            dtype = _mybir.dt.np(alloc.dtype)
            out_avals.append(jax.core.ShapedArray(shape, dtype))
            zero_outs.append(np.zeros(shape, dtype))
    n_params = len(in_names)
    n_outs = len(out_avals)
    in_names_all = in_names + out_names
    if partition_name is not None:
        in_names_all.append(partition_name)
    donate = tuple(range(n_params, n_params + n_outs))

    def _body(*args):
        operands = list(args)
        if partition_name is not None:
            operands.append(partition_id_tensor())
        return tuple(_bass_exec_p.bind(
            *operands, out_avals=tuple(out_avals),
            in_names=tuple(in_names_all), out_names=tuple(out_names),
            lowering_input_output_aliases=(), sim_require_finite=True,
            sim_require_nnan=True, nc=nc))

    devices = jax.devices()[:n_cores]
    mesh = Mesh(np.asarray(devices), ("core",))
    in_specs = (PartitionSpec("core"),) * (n_params + n_outs)
    out_specs = (PartitionSpec("core"),) * n_outs
    fn = jax.jit(shard_map(_body, mesh=mesh, in_specs=in_specs,
                           out_specs=out_specs, check_rep=False),
                 donate_argnums=donate, keep_unused=True)
    sh = NamedSharding(mesh, PartitionSpec("core"))
    concat_in = [
        jax.device_put(np.concatenate([np.asarray(in_maps[c][nm])
                                       for c in range(n_cores)], axis=0), sh)
        for nm in in_names]
    times = []
    for it in range(iters + 1):
        zs = [jax.device_put(
            np.zeros((n_cores * z.shape[0], *z.shape[1:]), z.dtype), sh)
            for z in zero_outs]
        for z in zs:
            z.block_until_ready()
        t0 = time.perf_counter()
        outs = fn(*concat_in, *zs)
        for o in outs:
            o.block_until_ready()
        dt = time.perf_counter() - t0
        if it > 0:  # skip warmup
            times.append(dt)
        del outs
    return float(np.median(times))
